# revision 44
# baseline (speedup 1.0000x reference)
"""Trainium2 Bass kernel for a 3x3 stride-1 pad-1 Conv2d (NCHW).

Problem (hardcoded): x (16, 128, 128, 128) f32, K (3, 3, 128, 256) f32.
The reference reinterprets K's flat buffer as (Cin, kh, kw, Cout) and only
writes output rows/cols 0..124 (the rest of the 128x128 output stays zero).

Strategy: data-parallel over batch (2 images/core on 8 cores) with a
vertical F(2,3) Winograd factorization. The 3 vertical taps collapse into
4 transform points j shared by each pair of output rows: the DVE (plus
GpSimd for X_3) computes X_j = B^T d over row pairs in bf16, the PE
contracts X_j with host-side G-transformed weights W_j over Cin=128
partitions, accumulating the 3 horizontal taps kw into one PSUM bank per
j (12 matmuls of N=500 per 8-row output tile-half instead of the direct
method's 18 -- a 1.5x PE-cycle reduction; ~217 ns/matmul measured, vs a
208 ns roofline at 2.4 GHz), then A^T (even rows = M0+M1+M2, odd rows =
M1-M2-M3) is applied on the way out. Measured ~183.6 us vs the 258 us
direct-conv bf16 baseline; PE ~97.5% busy.

The A^T drain is the scarce path: every engine may read at most one PSUM
operand per instruction (GpSimd none), and PSUM reads run at ~half rate
(~1.3-1.4 ns/lane-elem on Scalar/DVE vs 0.26 for the DVE's 2x bf16 SBUF
mode). So chains run in order j=3,1,2,0, the Scalar engine stages
M3/M1/M2 (and every other tile-half M0) to bf16 SBUF as each chain
retires, and the DVE finishes with at most one PSUM-touching op plus
three all-bf16 SBUF combines. Intermediates are bf16; accumulation stays
fp32 in PSUM (rel_l2 ~4.8e-3 vs the 2e-2 gate). Stores are bf16 (halves
store traffic); the host casts back to f32 and zeroes rows/cols >= 125.

The padded plane is 128x128 (rows/cols -1..126; original row/col 127
only feeds invalid outputs) and streams in row-band chunks, smallest
first. All 12 weight blocks load as ONE bulk DMA (786 KB, contiguous
6 KB per partition) -- this beat both per-block streaming and
consumption-order streaming by ~3 us; the tile scheduler is sensitive to
emission order, so change one thing at a time and re-measure. 28 dummy
matmuls on a zeroed tile keep the PE busy from t~1us until the first
real matmul (~7.5us) so the HAM clock gate ramps 1.2 -> 2.4 GHz with no
idle epoch: a PE-idle gap during ramp-up can leave the whole run at
2.0 GHz (~20% slower; this slow mode also strikes environmentally
regardless of kernel content).
"""

import ml_dtypes
import numpy as np

import concourse.bacc as bacc
import concourse.mybir as mybir
import concourse.tile as tile
from concourse.bass_utils import run_bass_kernel_spmd

N_CORES = 8
B, CIN, H, W = 16, 128, 128, 128
COUT = 256
BPC = B // N_CORES
HP = 128   # padded rows -1..126 (row 127 only feeds invalid outputs)
WPAD = 128  # padded cols -1..126 (col 127 only feeds invalid outputs)
VALID = 125
NRP = 63        # row pairs per image (out rows 0..125)
RP_PER_TILE = 4  # row pairs per output tile (8 output rows)
NTILES = 16      # 15 full tiles + 1 tile of 3 row pairs
F32 = mybir.dt.float32
BF16 = mybir.dt.bfloat16

# (first tile, n tiles) chunks of row-pair tiles; chunk input rows are
# 8*t0 .. 8*t0 + 8*nt + 1 (2-row overlap between chunks), capped at HP.
CHUNKS = [(0, 1), (1, 1), (2, 2), (4, 2), (6, 2), (8, 2), (10, 2), (12, 2), (14, 2)]
CHUNK_MAX_ROWS = 8 * 2 + 2

_NC_CACHE = {}


def _build_nc(reps=1):
    nc = bacc.Bacc()
    x_in = nc.dram_tensor("x", [BPC, CIN, HP, WPAD], BF16, kind="ExternalInput")
    w_in = nc.dram_tensor("w", [CIN, 12 * COUT], BF16, kind="ExternalInput")
    out_t = nc.dram_tensor("out", [BPC, COUT, H * W], BF16, kind="ExternalOutput")

    with tile.TileContext(nc) as tc:
        with (
            tc.tile_pool(name="wpool", bufs=1) as wpool,
            tc.tile_pool(name="dpool", bufs=1) as dpool,
            tc.tile_pool(name="xpool", bufs=3) as xpool,
            tc.tile_pool(name="tpool", bufs=3) as tpool,
            tc.tile_pool(name="mpool", bufs=3) as mpool,
            tc.tile_pool(name="opool", bufs=6) as opool,
            tc.tile_pool(name="pspool", bufs=8, space="PSUM") as pspool,
        ):
            w_sb = wpool.tile([CIN, 12 * COUT], BF16)
            # PE pre-warm: the HAM clock gate keeps the PE at 1.2 GHz until
            # ~3.4us of sustained matmul activity. The first real matmul
            # can't fire until w+chunk0+transform land; fill that shadow
            # with dummy matmuls on never-read SBUF so the real stream
            # starts at 2.4 GHz.
            d_x = dpool.tile([CIN, 512], BF16)
            d_ps = pspool.tile([128, 512], F32, tag="ps")
            nc.vector.memset(d_x[:], 0)
            for j in range(14):
                nc.tensor.matmul(
                    d_ps[:], d_x[:, 0:128], d_x[:], start=True, stop=True
                )

            nchunk = 0
            for b in [b for _ in range(reps) for b in range(BPC)]:
                for t0, ntile in CHUNKS:
                    nchunk += 1
                    r0 = 8 * t0
                    nrows = min(8 * ntile + 2, HP - r0)
                    nrp_c = (nrows - 2) // 2
                    xc = xpool.tile([CIN, CHUNK_MAX_ROWS, WPAD], BF16)
                    nc.sync.dma_start(
                        out=xc[:, 0:nrows, :],
                        in_=x_in[b, :, r0 : r0 + nrows, :],
                    )
                    if nchunk == 1:
                        # Weights in 4 contiguous segments, chain-consumption
                        # order (j=3,1 first), emitted after chunk0's x DMA.
                        for a, z in ((2304, 3072), (768, 1536), (1536, 2304), (0, 768)):
                            nc.sync.dma_start(
                                out=w_sb[:, a:z], in_=w_in[:, a:z]
                            )
                    # Input transform X_j = B^T d over row pairs, on GpSimd
                    # (SBUF-only there, and it keeps the DVE free for the
                    # PSUM-side output transform).
                    xt = tpool.tile([CIN, 4, CHUNK_MAX_ROWS // 2, WPAD], BF16)
                    n2 = 2 * nrp_c
                    d0 = xc[:, 0:n2:2, :]
                    d1 = xc[:, 1 : n2 + 1 : 2, :]
                    d2 = xc[:, 2 : n2 + 2 : 2, :]
                    d3 = xc[:, 3 : n2 + 2 : 2, :]
                    nc.vector.tensor_sub(xt[:, 0, 0:nrp_c, :], d0, d2)
                    nc.vector.tensor_add(xt[:, 1, 0:nrp_c, :], d1, d2)
                    nc.vector.tensor_sub(xt[:, 2, 0:nrp_c, :], d2, d1)
                    # GpSimd's ~2us op would gate the very first j=3 chain;
                    # keep the first two chunks' X3 on the (faster) DVE.
                    x3_eng = nc.vector if nchunk <= 2 else nc.gpsimd
                    x3_eng.tensor_sub(xt[:, 3, 0:nrp_c, :], d1, d3)
                    for lt in range(ntile):
                        t = t0 + lt
                        nrp = 3 if t == NTILES - 1 else RP_PER_TILE
                        lrp = RP_PER_TILE * lt
                        nr2 = 2 * nrp
                        for h in range(2):
                            ps = {}
                            # Output transform A^T M: even rows M0+M1+M2,
                            # odd rows M1-M2-M3. PSUM reads are the scarce
                            # resource (each instruction may read at most
                            # ONE PSUM operand, at ~half rate): run the
                            # chains in order j=3,1,2,0 and have the Scalar
                            # engine stage M3/M1/M2 to bf16 SBUF as each
                            # chain retires, leaving the DVE one PSUM-
                            # touching op (M0) plus three all-bf16 SBUF
                            # combines that run in its 2x mode.
                            cs = {}
                            # Alternate M0's PSUM staging between the Scalar
                            # engine (~70% busy) and the DVE so neither
                            # becomes the wall.
                            c0_on_act = (2 * t + h) % 2 == 0
                            for j in (3, 1, 2, 0):
                                ps[j] = pspool.tile(
                                    [128, RP_PER_TILE, VALID],
                                    F32,
                                    tag="ps",
                                    name=f"ps{j}",
                                )
                                for kw in range(3):
                                    c0 = (j * 3 + kw) * 256 + h * 128
                                    nc.tensor.matmul(
                                        ps[j][:, 0:nrp, :],
                                        w_sb[:, c0 : c0 + 128],
                                        xt[:, j, lrp : lrp + nrp, kw : kw + VALID],
                                        start=(kw == 0),
                                        stop=(kw == 2),
                                    )
                                if j != 0 or c0_on_act:
                                    cs[j] = mpool.tile(
                                        [128, RP_PER_TILE, VALID],
                                        BF16,
                                        tag=f"c{j}",
                                        name=f"c{j}",
                                    )
                                    nc.scalar.copy(
                                        out=cs[j][:, 0:nrp, :], in_=ps[j][:, 0:nrp, :]
                                    )
                            ta = mpool.tile([128, RP_PER_TILE, VALID], BF16, tag="ta")
                            tb = mpool.tile([128, RP_PER_TILE, VALID], BF16, tag="tb")
                            ob = opool.tile([128, 2 * RP_PER_TILE, W], BF16, tag="ob")
                            ev = ob[:, 0:nr2:2, 0:VALID]
                            od = ob[:, 1:nr2:2, 0:VALID]
                            c1v = cs[1][:, 0:nrp, :]
                            c2v = cs[2][:, 0:nrp, :]
                            c3v = cs[3][:, 0:nrp, :]
                            tav = ta[:, 0:nrp, :]
                            tbv = tb[:, 0:nrp, :]
                            nc.vector.tensor_sub(tbv, c1v, c2v)
                            nc.vector.tensor_sub(od, tbv, c3v)
                            if c0_on_act:
                                nc.vector.tensor_add(tav, c1v, cs[0][:, 0:nrp, :])
                            else:
                                nc.vector.tensor_add(tav, c1v, ps[0][:, 0:nrp, :])
                            nc.vector.tensor_add(ev, tav, c2v)
                            nc.sync.dma_start(
                                out=out_t[
                                    b,
                                    h * 128 : (h + 1) * 128,
                                    8 * t * W : (8 * t + nr2) * W,
                                ],
                                in_=ob[:, 0:nr2, :],
                            )
    nc.finalize()
    return nc


def _get_nc(reps=1):
    if reps not in _NC_CACHE:
        _NC_CACHE[reps] = _build_nc(reps)
    return _NC_CACHE[reps]


_G = np.array(
    [[1, 0, 0], [0.5, 0.5, 0.5], [0.5, -0.5, 0.5], [0, 0, 1]], dtype=np.float32
)


def _run(x, K, trace=False, reps=1):
    x_pad = np.zeros((B, CIN, HP, WPAD), dtype=ml_dtypes.bfloat16)
    x_pad[:, :, 1:HP, 1:WPAD] = np.asarray(x, dtype=np.float32)[
        :, :, 0 : HP - 1, 0 : WPAD - 1
    ].astype(ml_dtypes.bfloat16)
    # Reference reinterprets K's flat buffer as (Cin, kh, kw, Cout); fold
    # the Winograd G transform over kh on the host: W_j = sum_kh G[j,kh] w.
    w = np.asarray(K, dtype=np.float32).reshape(CIN, 3, 3, COUT)
    wj = np.einsum("jh,chwo->jcwo", _G, w)
    w_host = (
        np.ascontiguousarray(wj.transpose(1, 0, 2, 3))
        .reshape(CIN, 12 * COUT)
        .astype(ml_dtypes.bfloat16)
    )
    in_maps = [
        {"x": x_pad[i * BPC : (i + 1) * BPC], "w": w_host} for i in range(N_CORES)
    ]
    res = run_bass_kernel_spmd(
        _get_nc(reps), in_maps, list(range(N_CORES)), trace=trace
    )
    out = np.concatenate(
        [
            res.results[i]["out"]
            .reshape(BPC, COUT, H, W)
            .astype(np.float32)
            for i in range(N_CORES)
        ],
        axis=0,
    )
    out[:, :, VALID:, :] = 0
    out[:, :, :, VALID:] = 0
    return out, res


def kernel(x, K):
    out, _ = _run(x, K, trace=False)
    return out


# revision 45
# speedup vs baseline: 1.0301x; 1.0301x over previous
"""Trainium2 Bass kernel for a 3x3 stride-1 pad-1 Conv2d (NCHW).

Problem (hardcoded): x (16, 128, 128, 128) f32, K (3, 3, 128, 256) f32.
The reference reinterprets K's flat buffer as (Cin, kh, kw, Cout) and only
writes output rows/cols 0..124 (the rest of the 128x128 output stays zero).

Strategy: data-parallel over batch (2 images/core on 8 cores) with a
vertical F(2,3) Winograd factorization. The 3 vertical taps collapse into
4 transform points j shared by each pair of output rows: the DVE (plus
GpSimd for X_3) computes X_j = B^T d over row pairs in bf16, the PE
contracts X_j with host-side G-transformed weights W_j over Cin=128
partitions, accumulating the 3 horizontal taps kw into one PSUM bank per
j (12 matmuls of N=500 per 8-row output tile-half instead of the direct
method's 18 -- a 1.5x PE-cycle reduction; ~217 ns/matmul measured, vs a
208 ns roofline at 2.4 GHz), then A^T (even rows = M0+M1+M2, odd rows =
M1-M2-M3) is applied on the way out. Measured ~183.6 us vs the 258 us
direct-conv bf16 baseline; PE ~97.5% busy.

The A^T drain is the scarce path: every engine may read at most one PSUM
operand per instruction (GpSimd none), and PSUM reads run at ~half rate
(~1.3-1.4 ns/lane-elem on Scalar/DVE vs 0.26 for the DVE's 2x bf16 SBUF
mode). So chains run in order j=3,1,2,0, the Scalar engine stages
M3/M1/M2 (and every other tile-half M0) to bf16 SBUF as each chain
retires, and the DVE finishes with at most one PSUM-touching op plus
three all-bf16 SBUF combines. Intermediates are bf16; accumulation stays
fp32 in PSUM (rel_l2 ~4.8e-3 vs the 2e-2 gate). Stores are bf16 (halves
store traffic); the host casts back to f32 and zeroes rows/cols >= 125.

The padded plane is 128x128 (rows/cols -1..126; original row/col 127
only feeds invalid outputs) and streams in row-band chunks, smallest
first. All 12 weight blocks load as ONE bulk DMA (786 KB, contiguous
6 KB per partition) -- this beat both per-block streaming and
consumption-order streaming by ~3 us; the tile scheduler is sensitive to
emission order, so change one thing at a time and re-measure. 28 dummy
matmuls on a zeroed tile keep the PE busy from t~1us until the first
real matmul (~7.5us) so the HAM clock gate ramps 1.2 -> 2.4 GHz with no
idle epoch: a PE-idle gap during ramp-up can leave the whole run at
2.0 GHz (~20% slower; this slow mode also strikes environmentally
regardless of kernel content).
"""

import ml_dtypes
import numpy as np

import concourse.bacc as bacc
import concourse.mybir as mybir
import concourse.tile as tile
from concourse.bass_utils import run_bass_kernel_spmd

N_CORES = 8
B, CIN, H, W = 16, 128, 128, 128
COUT = 256
BPC = B // N_CORES
HP = 128   # padded rows -1..126 (row 127 only feeds invalid outputs)
WPAD = 128  # padded cols -1..126 (col 127 only feeds invalid outputs)
VALID = 125
NRP = 63        # row pairs per image (out rows 0..125)
RP_PER_TILE = 4  # row pairs per output tile (8 output rows)
NTILES = 16      # 15 full tiles + 1 tile of 3 row pairs
F32 = mybir.dt.float32
BF16 = mybir.dt.bfloat16

# (first tile, n tiles) chunks of row-pair tiles; chunk input rows are
# 8*t0 .. 8*t0 + 8*nt + 1 (2-row overlap between chunks), capped at HP.
CHUNKS = [(0, 1), (1, 1), (2, 2), (4, 2), (6, 2), (8, 2), (10, 2), (12, 2), (14, 2)]
CHUNK_MAX_ROWS = 8 * 2 + 2

_NC_CACHE = {}


def _build_nc(reps=1):
    nc = bacc.Bacc()
    x_in = nc.dram_tensor("x", [BPC, CIN, HP, WPAD], BF16, kind="ExternalInput")
    w_in = nc.dram_tensor("w", [CIN, 12 * COUT], BF16, kind="ExternalInput")
    out_t = nc.dram_tensor("out", [BPC, COUT, H * W], BF16, kind="ExternalOutput")

    with tile.TileContext(nc) as tc:
        with (
            tc.tile_pool(name="wpool", bufs=1) as wpool,
            tc.tile_pool(name="dpool", bufs=1) as dpool,
            tc.tile_pool(name="xpool", bufs=3) as xpool,
            tc.tile_pool(name="tpool", bufs=3) as tpool,
            tc.tile_pool(name="mpool", bufs=3) as mpool,
            tc.tile_pool(name="opool", bufs=6) as opool,
            tc.tile_pool(name="pspool", bufs=8, space="PSUM") as pspool,
        ):
            w_sb = wpool.tile([CIN, 12 * COUT], BF16)
            # PE pre-warm: the HAM clock gate keeps the PE at 1.2 GHz until
            # ~3.4us of sustained matmul activity. The first real matmul
            # can't fire until w+chunk0+transform land; fill that shadow
            # with dummy matmuls on never-read SBUF so the real stream
            # starts at 2.4 GHz.
            d_x = dpool.tile([CIN, 512], BF16)
            d_ps = pspool.tile([128, 512], F32, tag="ps")
            nc.vector.memset(d_x[:], 0)
            for j in range(18):
                nc.tensor.matmul(
                    d_ps[:], d_x[:, 0:128], d_x[:], start=True, stop=True
                )

            nchunk = 0
            for b in [b for _ in range(reps) for b in range(BPC)]:
                for t0, ntile in CHUNKS:
                    nchunk += 1
                    r0 = 8 * t0
                    nrows = min(8 * ntile + 2, HP - r0)
                    nrp_c = (nrows - 2) // 2
                    xc = xpool.tile([CIN, CHUNK_MAX_ROWS, WPAD], BF16)
                    nc.sync.dma_start(
                        out=xc[:, 0:nrows, :],
                        in_=x_in[b, :, r0 : r0 + nrows, :],
                    )
                    if nchunk == 1:
                        # All 12 weight blocks in one bulk DMA, emitted after
                        # chunk0's x DMA so the transforms start sooner.
                        nc.sync.dma_start(out=w_sb[:], in_=w_in[:])
                    # Input transform X_j = B^T d over row pairs, on GpSimd
                    # (SBUF-only there, and it keeps the DVE free for the
                    # PSUM-side output transform).
                    xt = tpool.tile([CIN, 4, CHUNK_MAX_ROWS // 2, WPAD], BF16)
                    n2 = 2 * nrp_c
                    d0 = xc[:, 0:n2:2, :]
                    d1 = xc[:, 1 : n2 + 1 : 2, :]
                    d2 = xc[:, 2 : n2 + 2 : 2, :]
                    d3 = xc[:, 3 : n2 + 2 : 2, :]
                    nc.vector.tensor_sub(xt[:, 0, 0:nrp_c, :], d0, d2)
                    nc.vector.tensor_add(xt[:, 1, 0:nrp_c, :], d1, d2)
                    nc.vector.tensor_sub(xt[:, 2, 0:nrp_c, :], d2, d1)
                    # GpSimd's ~2us op would gate the very first j=3 chain;
                    # keep the first two chunks' X3 on the (faster) DVE.
                    x3_eng = nc.vector if nchunk <= 2 else nc.gpsimd
                    x3_eng.tensor_sub(xt[:, 3, 0:nrp_c, :], d1, d3)
                    for lt in range(ntile):
                        t = t0 + lt
                        nrp = 3 if t == NTILES - 1 else RP_PER_TILE
                        lrp = RP_PER_TILE * lt
                        nr2 = 2 * nrp
                        for h in range(2):
                            ps = {}
                            # Output transform A^T M: even rows M0+M1+M2,
                            # odd rows M1-M2-M3. PSUM reads are the scarce
                            # resource (each instruction may read at most
                            # ONE PSUM operand, at ~half rate): run the
                            # chains in order j=3,1,2,0 and have the Scalar
                            # engine stage M3/M1/M2 to bf16 SBUF as each
                            # chain retires, leaving the DVE one PSUM-
                            # touching op (M0) plus three all-bf16 SBUF
                            # combines that run in its 2x mode.
                            cs = {}
                            # Alternate M0's PSUM staging between the Scalar
                            # engine (~70% busy) and the DVE so neither
                            # becomes the wall.
                            c0_on_act = (2 * t + h) % 2 == 0
                            for j in (3, 1, 2, 0):
                                ps[j] = pspool.tile(
                                    [128, RP_PER_TILE, VALID],
                                    F32,
                                    tag="ps",
                                    name=f"ps{j}",
                                )
                                for kw in range(3):
                                    c0 = (j * 3 + kw) * 256 + h * 128
                                    nc.tensor.matmul(
                                        ps[j][:, 0:nrp, :],
                                        w_sb[:, c0 : c0 + 128],
                                        xt[:, j, lrp : lrp + nrp, kw : kw + VALID],
                                        start=(kw == 0),
                                        stop=(kw == 2),
                                    )
                                if j != 0 or c0_on_act:
                                    cs[j] = mpool.tile(
                                        [128, RP_PER_TILE, VALID],
                                        BF16,
                                        tag=f"c{j}",
                                        name=f"c{j}",
                                    )
                                    nc.scalar.copy(
                                        out=cs[j][:, 0:nrp, :], in_=ps[j][:, 0:nrp, :]
                                    )
                            ta = mpool.tile([128, RP_PER_TILE, VALID], BF16, tag="ta")
                            tb = mpool.tile([128, RP_PER_TILE, VALID], BF16, tag="tb")
                            ob = opool.tile([128, 2 * RP_PER_TILE, W], BF16, tag="ob")
                            ev = ob[:, 0:nr2:2, 0:VALID]
                            od = ob[:, 1:nr2:2, 0:VALID]
                            c1v = cs[1][:, 0:nrp, :]
                            c2v = cs[2][:, 0:nrp, :]
                            c3v = cs[3][:, 0:nrp, :]
                            tav = ta[:, 0:nrp, :]
                            tbv = tb[:, 0:nrp, :]
                            nc.vector.tensor_sub(tbv, c1v, c2v)
                            nc.vector.tensor_sub(od, tbv, c3v)
                            if c0_on_act:
                                nc.vector.tensor_add(tav, c1v, cs[0][:, 0:nrp, :])
                            else:
                                nc.vector.tensor_add(tav, c1v, ps[0][:, 0:nrp, :])
                            nc.vector.tensor_add(ev, tav, c2v)
                            nc.sync.dma_start(
                                out=out_t[
                                    b,
                                    h * 128 : (h + 1) * 128,
                                    8 * t * W : (8 * t + nr2) * W,
                                ],
                                in_=ob[:, 0:nr2, :],
                            )
    nc.finalize()
    return nc


def _get_nc(reps=1):
    if reps not in _NC_CACHE:
        _NC_CACHE[reps] = _build_nc(reps)
    return _NC_CACHE[reps]


_G = np.array(
    [[1, 0, 0], [0.5, 0.5, 0.5], [0.5, -0.5, 0.5], [0, 0, 1]], dtype=np.float32
)


def _run(x, K, trace=False, reps=1):
    x_pad = np.zeros((B, CIN, HP, WPAD), dtype=ml_dtypes.bfloat16)
    x_pad[:, :, 1:HP, 1:WPAD] = np.asarray(x, dtype=np.float32)[
        :, :, 0 : HP - 1, 0 : WPAD - 1
    ].astype(ml_dtypes.bfloat16)
    # Reference reinterprets K's flat buffer as (Cin, kh, kw, Cout); fold
    # the Winograd G transform over kh on the host: W_j = sum_kh G[j,kh] w.
    w = np.asarray(K, dtype=np.float32).reshape(CIN, 3, 3, COUT)
    wj = np.einsum("jh,chwo->jcwo", _G, w)
    w_host = (
        np.ascontiguousarray(wj.transpose(1, 0, 2, 3))
        .reshape(CIN, 12 * COUT)
        .astype(ml_dtypes.bfloat16)
    )
    in_maps = [
        {"x": x_pad[i * BPC : (i + 1) * BPC], "w": w_host} for i in range(N_CORES)
    ]
    res = run_bass_kernel_spmd(
        _get_nc(reps), in_maps, list(range(N_CORES)), trace=trace
    )
    out = np.concatenate(
        [
            res.results[i]["out"]
            .reshape(BPC, COUT, H, W)
            .astype(np.float32)
            for i in range(N_CORES)
        ],
        axis=0,
    )
    out[:, :, VALID:, :] = 0
    out[:, :, :, VALID:] = 0
    return out, res


def kernel(x, K):
    out, _ = _run(x, K, trace=False)
    return out
